# revision 13
# baseline (speedup 1.0000x reference)
"""Trainium2 Bass kernel for a dense-gated MoE (DGMoE) block.

Strategy (expert-parallel over 8 NeuronCores):
  - Core k receives expert k's weights (W1[k], b1[k], W2[k]) and a one-hot
    selector; every core redundantly computes the (cheap) fp32 gating
    pipeline: logits = x@Wt + prev@Wgt, st = softmax, mask = st > thr,
    w_k = (st*mask)[.,k] / (sum(st*mask) + 1e-6).
  - The big FFN (x@W1 -> gelu -> @W2) runs in bf16 with fp32 PSUM
    accumulation; each core produces its expert's weighted partial
    y_k = w_k * (gelu(x@W1+b1)@W2), and the host sums the 8 partials.
  - b2's contribution is linear in the gating weights, so it's added on the
    host: y += w @ b2 (exact for the device-side mask).

Layouts: x and prev_logits are passed pre-transposed ([D, B*T] / [K, B*T])
so the contraction dim lands on SBUF partitions without on-device
transposes. logits/mask are returned in a kernel-private [128, 32*8] tiling
that the host unscrambles.
"""

import os

import numpy as np
import ml_dtypes

import concourse.bass as bass
import concourse.mybir as mybir
import concourse.tile as tile
from concourse import bacc
from concourse.bass import ds, ts
from concourse.bass_utils import run_bass_kernel_spmd

BF16 = ml_dtypes.bfloat16
F32 = np.float32

B, T, D = 2, 2048, 1024
K, F = 8, 4096
BT = B * T
LAMBDA = 0.5
NCORES = 8

TOK = 512            # FFN token tile
NT = BT // TOK       # 8 token tiles
DC = D // 128        # 8 contraction chunks for mm1
FC = F // 128        # 32 f chunks
GJ = BT // 128       # 32 gating token chunks
DT_F32 = mybir.dt.float32
DT_BF16 = mybir.dt.bfloat16

AF = mybir.ActivationFunctionType
ALU = mybir.AluOpType
AX = mybir.AxisListType

LAST_RESULT = None  # BassKernelResults of the most recent run (for test harness)


def _build(repeat=1, w1bufs=6, xbbufs=2, xgbufs=2, ybufs=3, phbufs=3):
    nc = bacc.Bacc(
        "TRN2", target_bir_lowering=False, debug=False, enable_asserts=False
    )

    # ---- I/O ----
    xtf = nc.dram_tensor("xtf", [D, BT], DT_F32, kind="ExternalInput")
    xtb = nc.dram_tensor("xtb", [D, BT], DT_BF16, kind="ExternalInput")
    pt = nc.dram_tensor("pt", [K, BT], DT_F32, kind="ExternalInput")
    wt = nc.dram_tensor("wt", [D, K], DT_F32, kind="ExternalInput")
    wgt = nc.dram_tensor("wgt", [K, K], DT_F32, kind="ExternalInput")
    thr = nc.dram_tensor("thr", [128, GJ * K], DT_F32, kind="ExternalInput")
    sel = nc.dram_tensor("sel", [128, GJ * K], DT_F32, kind="ExternalInput")
    w1 = nc.dram_tensor("w1", [D, F], DT_BF16, kind="ExternalInput")
    b1t = nc.dram_tensor("b1t", [128, FC], DT_F32, kind="ExternalInput")
    w2 = nc.dram_tensor("w2", [F, D], DT_BF16, kind="ExternalInput")

    y = nc.dram_tensor("y", [BT, D], DT_F32, kind="ExternalOutput")
    lo = nc.dram_tensor("lo", [128, GJ * K], DT_F32, kind="ExternalOutput")
    mo = nc.dram_tensor("mo", [128, GJ * K], DT_F32, kind="ExternalOutput")

    with tile.TileContext(nc) as tc:
        with (
            tc.tile_pool(name="const", bufs=1) as cpool,
            tc.tile_pool(name="w1p", bufs=w1bufs) as w1pool,
            tc.tile_pool(name="xbp", bufs=xbbufs) as xbpool,
            tc.tile_pool(name="xgp", bufs=xgbufs) as xgpool,
            tc.tile_pool(name="ptp", bufs=2) as ptpool,
            tc.tile_pool(name="hp", bufs=1) as hpool,
            tc.tile_pool(name="yp", bufs=ybufs) as ypool,
            tc.tile_pool(name="gp", bufs=1) as gpool,
            tc.tile_pool(name="psh", bufs=phbufs, space="PSUM") as pspool,
            tc.tile_pool(name="pso", bufs=2, space="PSUM") as popool,
            tc.tile_pool(name="psg", bufs=1, space="PSUM") as pgpool,
        ):
            # ---- resident constants ----
            wt_s = cpool.tile([128, DC, K], DT_F32, name="wt_s")
            nc.sync.dma_start(
                out=wt_s, in_=wt.ap().rearrange("(c p) k -> p c k", p=128)
            )
            wgt_s = cpool.tile([K, K], DT_F32, name="wgt_s")
            nc.sync.dma_start(out=wgt_s, in_=wgt.ap())
            thr_s = cpool.tile([128, GJ * K], DT_F32, name="thr_s")
            nc.sync.dma_start(out=thr_s, in_=thr.ap())
            sel_s = cpool.tile([128, GJ * K], DT_F32, name="sel_s")
            nc.sync.dma_start(out=sel_s, in_=sel.ap())
            b1s = cpool.tile([128, FC], DT_F32, name="b1s")
            nc.sync.dma_start(out=b1s, in_=b1t.ap())

            w2s = []
            for f in range(FC):
                w2t = cpool.tile(
                    [128, D], DT_BF16, tag=f"w2_{f}", name=f"w2_{f}"
                )
                nc.sync.dma_start(out=w2t, in_=w2.ap()[ts(f, 128), :])
                w2s.append(w2t)

            def body():
                if int(os.environ.get("KERNEL_NOGATE", "0")):
                    wsel_c = gpool.tile([128, GJ], DT_F32, name="wsel_c")
                    nc.vector.memset(wsel_c, 0.25)
                    _ffn(wsel_c)
                    return
                # ---- gating (fp32, all tokens batched as [128, GJ, K]) ----
                lpsum = pgpool.tile([128, GJ * K], DT_F32, name="lpsum")
                xtf_v = xtf.ap().rearrange("(c p) t -> p c t", p=128)
                for j in range(GJ):
                    xg = xgpool.tile([128, DC, 128], DT_F32, tag="xg", name="xg")
                    nc.sync.dma_start(out=xg, in_=xtf_v[:, :, ts(j, 128)])
                    ptj = ptpool.tile([K, 128], DT_F32, tag="ptj", name="ptj")
                    nc.sync.dma_start(out=ptj, in_=pt.ap()[:, ts(j, 128)])
                    ps_j = lpsum[:, ts(j, K)]
                    for c in range(DC):
                        nc.tensor.matmul(
                            ps_j,
                            lhsT=xg[:, c, :],
                            rhs=wt_s[:, c, :],
                            start=(c == 0),
                            stop=False,
                        )
                    nc.tensor.matmul(
                        ps_j, lhsT=ptj, rhs=wgt_s, start=False, stop=True
                    )

                lt = gpool.tile([128, GJ * K], DT_F32, name="lt")
                nc.vector.tensor_copy(out=lt, in_=lpsum)
                nc.sync.dma_start(out=lo.ap(), in_=lt)
                # softmax over K without max-subtraction (|logits| <~ 4 here)
                ex = gpool.tile([128, GJ * K], DT_F32, name="ex")
                nc.scalar.activation(out=ex, in_=lt, func=AF.Exp)
                ex3 = ex.rearrange("p (j k) -> p j k", k=K)
                sm = gpool.tile([128, GJ], DT_F32, name="sm")
                nc.vector.tensor_reduce(out=sm, in_=ex3, axis=AX.X, op=ALU.add)
                sminv = gpool.tile([128, GJ], DT_F32, name="sminv")
                nc.vector.reciprocal(out=sminv, in_=sm)
                st = gpool.tile([128, GJ * K], DT_F32, name="st")
                nc.vector.tensor_tensor(
                    out=st.rearrange("p (j k) -> p j k", k=K),
                    in0=ex3,
                    in1=sminv[:, :, None].broadcast_to([128, GJ, K]),
                    op=ALU.mult,
                )
                msk = gpool.tile([128, GJ * K], DT_F32, name="msk")
                nc.vector.tensor_tensor(out=msk, in0=st, in1=thr_s, op=ALU.is_gt)
                nc.sync.dma_start(out=mo.ap(), in_=msk)
                gg = gpool.tile([128, GJ * K], DT_F32, name="gg")
                nc.vector.tensor_mul(out=gg, in0=st, in1=msk)
                gs = gpool.tile([128, GJ], DT_F32, name="gs")
                nc.vector.tensor_reduce(
                    out=gs,
                    in_=gg.rearrange("p (j k) -> p j k", k=K),
                    axis=AX.X,
                    op=ALU.add,
                )
                gsel = gpool.tile([128, GJ * K], DT_F32, name="gsel")
                nc.vector.tensor_mul(out=gsel, in0=gg, in1=sel_s)
                wnum = gpool.tile([128, GJ], DT_F32, name="wnum")
                nc.vector.tensor_reduce(
                    out=wnum,
                    in_=gsel.rearrange("p (j k) -> p j k", k=K),
                    axis=AX.X,
                    op=ALU.add,
                )
                dn = gpool.tile([128, GJ], DT_F32, name="dn")
                nc.vector.tensor_scalar_add(out=dn, in0=gs, scalar1=1e-6)
                dninv = gpool.tile([128, GJ], DT_F32, name="dninv")
                nc.vector.reciprocal(out=dninv, in_=dn)
                wsel = gpool.tile([128, GJ], DT_F32, name="wsel")
                nc.vector.tensor_mul(out=wsel, in0=wnum, in1=dninv)
                _ffn(wsel)

            def _ffn(wsel):
                # ---- FFN: per 512-token tile, mm1+gelu then mm2+scale ----
                xtb_v = xtb.ap().rearrange("(c p) t -> p c t", p=128)
                w1_v = w1.ap().rearrange("(c p) f -> p c f", p=128)
                for ti in range(NT):
                    xb = xbpool.tile([128, DC, TOK], DT_BF16, tag="xb", name="xb")
                    nc.sync.dma_start(out=xb, in_=xtb_v[:, :, ds(ti * TOK, TOK)])
                    hts = []
                    for f in range(FC):
                        w1t = w1pool.tile(
                            [128, DC, 128], DT_BF16, tag="w1t", name="w1t"
                        )
                        nc.sync.dma_start(out=w1t, in_=w1_v[:, :, ts(f, 128)])
                        ph = pspool.tile([128, TOK], DT_F32, tag="ph", name="ph")
                        for c in range(DC):
                            nc.tensor.matmul(
                                ph,
                                lhsT=w1t[:, c, :],
                                rhs=xb[:, c, :],
                                start=(c == 0),
                                stop=(c == DC - 1),
                            )
                        ht = hpool.tile(
                            [128, TOK], DT_BF16, tag=f"ht{f}", name=f"ht{f}"
                        )
                        nc.scalar.activation(
                            out=ht,
                            in_=ph,
                            func=AF.Gelu,
                            bias=b1s[:, f : f + 1],
                            scale=1.0,
                        )
                        hts.append(ht)
                    for tj in range(TOK // 128):
                        j = ti * (TOK // 128) + tj
                        po0 = popool.tile([128, 512], DT_F32, tag="po0", name="po0")
                        po1 = popool.tile([128, 512], DT_F32, tag="po1", name="po1")
                        for f in range(FC):
                            lh = hts[f][:, ts(tj, 128)]
                            nc.tensor.matmul(
                                po0,
                                lhsT=lh,
                                rhs=w2s[f][:, 0:512],
                                start=(f == 0),
                                stop=(f == FC - 1),
                            )
                            nc.tensor.matmul(
                                po1,
                                lhsT=lh,
                                rhs=w2s[f][:, 512:1024],
                                start=(f == 0),
                                stop=(f == FC - 1),
                            )
                        yt0 = ypool.tile([128, 512], DT_F32, tag="yt0", name="yt0")
                        nc.vector.tensor_scalar_mul(
                            out=yt0, in0=po0, scalar1=wsel[:, j : j + 1]
                        )
                        nc.sync.dma_start(
                            out=y.ap()[ds(j * 128, 128), 0:512], in_=yt0
                        )
                        yt1 = ypool.tile([128, 512], DT_F32, tag="yt1", name="yt1")
                        nc.vector.tensor_scalar_mul(
                            out=yt1, in0=po1, scalar1=wsel[:, j : j + 1]
                        )
                        nc.sync.dma_start(
                            out=y.ap()[ds(j * 128, 128), 512:1024], in_=yt1
                        )

            if repeat == 1:
                body()
            else:
                with tc.For_i(0, repeat, 1):
                    body()

    nc.compile()
    return nc


_NC_CACHE = None


def _get_nc():
    global _NC_CACHE
    if _NC_CACHE is None:
        _NC_CACHE = _build(repeat=int(os.environ.get("KERNEL_REPEAT", "1")))
    return _NC_CACHE


def _prep_in_maps(x, prev_logits, Wt, Wgt, We_logits, W1, b1, W2, b2):
    # ---- host-side shard prep ----
    x2 = x.reshape(BT, D)
    xtf = np.ascontiguousarray(x2.T)
    xtb = np.ascontiguousarray(xtf.astype(BF16))
    pt = np.ascontiguousarray(prev_logits.reshape(BT, K).T)
    th = (F32(LAMBDA) / (F32(1.0) + np.exp(-We_logits, dtype=F32))).astype(F32)
    thr_tiled = np.ascontiguousarray(np.tile(th, (128, GJ)))

    shared = {
        "xtf": xtf,
        "xtb": xtb,
        "pt": pt,
        "wt": Wt,
        "wgt": Wgt,
        "thr": thr_tiled,
    }
    in_maps = []
    for k in range(NCORES):
        e = np.zeros(K, F32)
        e[k] = 1.0
        in_maps.append(
            dict(
                shared,
                sel=np.ascontiguousarray(np.tile(e, (128, GJ))),
                w1=np.ascontiguousarray(W1[k].astype(BF16)),
                b1t=np.ascontiguousarray(b1[k].reshape(FC, 128).T),
                w2=np.ascontiguousarray(W2[k].astype(BF16)),
            )
        )
    return in_maps


def _postprocess(results, b2):
    # ---- host-side unshard ----
    y = results[0]["y"].astype(F32)
    for k in range(1, NCORES):
        y = y + results[k]["y"]

    def untile(a):  # [128, GJ*K] -> [BT, K]
        return a.reshape(128, GJ, K).transpose(1, 0, 2).reshape(BT, K)

    logits = untile(results[0]["lo"])
    mask = untile(results[0]["mo"])

    # b2 enters linearly through the gating weights; add it on the host.
    e = np.exp(logits, dtype=F32)
    st = e / e.sum(-1, keepdims=True)
    g = st * mask
    w = g / (g.sum(-1, keepdims=True) + F32(1e-6))
    y = y + w @ b2

    sel_counts = mask.sum(axis=0, dtype=F32)
    avg_density = F32(sel_counts.sum() / F32(BT))

    return (
        y.reshape(B, T, D),
        logits.reshape(B, T, K),
        sel_counts,
        np.asarray(avg_density, F32),
    )


def _enable_jax_cache():
    try:
        import jax

        jax.config.update("jax_compilation_cache_dir", "/tmp/jax_kernel_cache")
        jax.config.update("jax_persistent_cache_min_compile_time_secs", 1.0)
    except Exception:
        pass


def kernel(x, prev_logits, Wt, Wgt, We_logits, W1, b1, W2, b2):
    global LAST_RESULT
    _enable_jax_cache()
    x = np.asarray(x, F32)
    prev_logits = np.asarray(prev_logits, F32)
    Wt = np.asarray(Wt, F32)
    Wgt = np.asarray(Wgt, F32)
    We_logits = np.asarray(We_logits, F32)
    W1 = np.asarray(W1, F32)
    b1 = np.asarray(b1, F32)
    W2 = np.asarray(W2, F32)
    b2 = np.asarray(b2, F32)

    in_maps = _prep_in_maps(x, prev_logits, Wt, Wgt, We_logits, W1, b1, W2, b2)
    nc = _get_nc()
    res = run_bass_kernel_spmd(nc, in_maps, core_ids=list(range(NCORES)))
    LAST_RESULT = res
    return _postprocess(res.results, b2)


# revision 14
# speedup vs baseline: 82.3782x; 82.3782x over previous
"""Trainium2 Bass kernel for a dense-gated MoE (DGMoE) block.

Strategy (expert-parallel over 8 NeuronCores):
  - Core k receives expert k's weights (W1[k], b1[k], W2[k]) and a one-hot
    selector; every core redundantly computes the (cheap) fp32 gating
    pipeline: logits = x@Wt + prev@Wgt, st = softmax, mask = st > thr,
    w_k = (st*mask)[.,k] / (sum(st*mask) + 1e-6).
  - The big FFN (x@W1 -> gelu -> @W2) runs in bf16 with fp32 PSUM
    accumulation; each core produces its expert's weighted partial
    y_k = w_k * (gelu(x@W1+b1)@W2), and the host sums the 8 partials.
  - b2's contribution is linear in the gating weights, so it's added on the
    host: y += w @ b2 (exact for the device-side mask).

Layouts: x and prev_logits are passed pre-transposed ([D, B*T] / [K, B*T])
so the contraction dim lands on SBUF partitions without on-device
transposes. logits/mask are returned in a kernel-private [128, 32*8] tiling
that the host unscrambles.
"""

import os

import numpy as np
import ml_dtypes

import concourse.bass as bass
import concourse.mybir as mybir
import concourse.tile as tile
from concourse import bacc
from concourse.bass import ds, ts
from concourse.bass_utils import run_bass_kernel_spmd

BF16 = ml_dtypes.bfloat16
F32 = np.float32

B, T, D = 2, 2048, 1024
K, F = 8, 4096
BT = B * T
LAMBDA = 0.5
NCORES = 8

TOK = 512            # FFN token tile
NT = BT // TOK       # 8 token tiles
DC = D // 128        # 8 contraction chunks for mm1
FC = F // 128        # 32 f chunks
GJ = BT // 128       # 32 gating token chunks
DT_F32 = mybir.dt.float32
DT_BF16 = mybir.dt.bfloat16

AF = mybir.ActivationFunctionType
ALU = mybir.AluOpType
AX = mybir.AxisListType

LAST_RESULT = None  # BassKernelResults of the most recent run (for test harness)


def _build(repeat=1, w1bufs=6, xbbufs=2, xgbufs=2, ybufs=3, phbufs=3):
    nc = bacc.Bacc(
        "TRN2", target_bir_lowering=False, debug=False, enable_asserts=False
    )

    # ---- I/O ----
    xtf = nc.dram_tensor("xtf", [D, BT], DT_F32, kind="ExternalInput")
    xtb = nc.dram_tensor("xtb", [D, BT], DT_BF16, kind="ExternalInput")
    pt = nc.dram_tensor("pt", [K, BT], DT_F32, kind="ExternalInput")
    wt = nc.dram_tensor("wt", [D, K], DT_F32, kind="ExternalInput")
    wgt = nc.dram_tensor("wgt", [K, K], DT_F32, kind="ExternalInput")
    thr = nc.dram_tensor("thr", [128, GJ * K], DT_F32, kind="ExternalInput")
    sel = nc.dram_tensor("sel", [128, GJ * K], DT_F32, kind="ExternalInput")
    w1 = nc.dram_tensor("w1", [D, F], DT_BF16, kind="ExternalInput")
    b1t = nc.dram_tensor("b1t", [128, FC], DT_F32, kind="ExternalInput")
    w2 = nc.dram_tensor("w2", [F, D], DT_BF16, kind="ExternalInput")

    y = nc.dram_tensor("y", [BT, D], DT_F32, kind="ExternalOutput")
    lo = nc.dram_tensor("lo", [128, GJ * K], DT_F32, kind="ExternalOutput")
    mo = nc.dram_tensor("mo", [128, GJ * K], DT_F32, kind="ExternalOutput")

    with tile.TileContext(nc) as tc:
        with (
            tc.tile_pool(name="const", bufs=1) as cpool,
            tc.tile_pool(name="w1p", bufs=w1bufs) as w1pool,
            tc.tile_pool(name="xbp", bufs=xbbufs) as xbpool,
            tc.tile_pool(name="xgp", bufs=xgbufs) as xgpool,
            tc.tile_pool(name="ptp", bufs=2) as ptpool,
            tc.tile_pool(name="hp", bufs=1) as hpool,
            tc.tile_pool(name="yp", bufs=ybufs) as ypool,
            tc.tile_pool(name="gp", bufs=1) as gpool,
            tc.tile_pool(name="psh", bufs=phbufs, space="PSUM") as pspool,
            tc.tile_pool(name="pso", bufs=2, space="PSUM") as popool,
            tc.tile_pool(name="psg", bufs=1, space="PSUM") as pgpool,
        ):
            # ---- resident constants ----
            wt_s = cpool.tile([128, DC, K], DT_F32, name="wt_s")
            nc.sync.dma_start(
                out=wt_s, in_=wt.ap().rearrange("(c p) k -> p c k", p=128)
            )
            wgt_s = cpool.tile([K, K], DT_F32, name="wgt_s")
            nc.sync.dma_start(out=wgt_s, in_=wgt.ap())
            thr_s = cpool.tile([128, GJ * K], DT_F32, name="thr_s")
            nc.sync.dma_start(out=thr_s, in_=thr.ap())
            sel_s = cpool.tile([128, GJ * K], DT_F32, name="sel_s")
            nc.sync.dma_start(out=sel_s, in_=sel.ap())
            b1s = cpool.tile([128, FC], DT_F32, name="b1s")
            nc.sync.dma_start(out=b1s, in_=b1t.ap())

            w2s = []
            for f in range(FC):
                w2t = cpool.tile(
                    [128, D], DT_BF16, tag=f"w2_{f}", name=f"w2_{f}"
                )
                nc.sync.dma_start(out=w2t, in_=w2.ap()[ts(f, 128), :])
                w2s.append(w2t)

            def body():
                if int(os.environ.get("KERNEL_NOGATE", "0")):
                    wsel_c = gpool.tile([128, GJ], DT_F32, name="wsel_c")
                    nc.vector.memset(wsel_c, 0.25)
                    _ffn(wsel_c)
                    return
                # ---- gating (fp32, all tokens batched as [128, GJ, K]) ----
                lpsum = pgpool.tile([128, GJ * K], DT_F32, name="lpsum")
                xtf_v = xtf.ap().rearrange("(c p) t -> p c t", p=128)
                for j in range(GJ):
                    xg = xgpool.tile([128, DC, 128], DT_F32, tag="xg", name="xg")
                    nc.sync.dma_start(out=xg, in_=xtf_v[:, :, ts(j, 128)])
                    ptj = ptpool.tile([K, 128], DT_F32, tag="ptj", name="ptj")
                    nc.sync.dma_start(out=ptj, in_=pt.ap()[:, ts(j, 128)])
                    ps_j = lpsum[:, ts(j, K)]
                    for c in range(DC):
                        nc.tensor.matmul(
                            ps_j,
                            lhsT=xg[:, c, :],
                            rhs=wt_s[:, c, :],
                            start=(c == 0),
                            stop=False,
                        )
                    nc.tensor.matmul(
                        ps_j, lhsT=ptj, rhs=wgt_s, start=False, stop=True
                    )

                lt = gpool.tile([128, GJ * K], DT_F32, name="lt")
                nc.vector.tensor_copy(out=lt, in_=lpsum)
                nc.sync.dma_start(out=lo.ap(), in_=lt)
                # softmax over K without max-subtraction (|logits| <~ 4 here)
                ex = gpool.tile([128, GJ * K], DT_F32, name="ex")
                nc.scalar.activation(out=ex, in_=lt, func=AF.Exp)
                ex3 = ex.rearrange("p (j k) -> p j k", k=K)
                sm = gpool.tile([128, GJ], DT_F32, name="sm")
                nc.vector.tensor_reduce(out=sm, in_=ex3, axis=AX.X, op=ALU.add)
                sminv = gpool.tile([128, GJ], DT_F32, name="sminv")
                nc.vector.reciprocal(out=sminv, in_=sm)
                st = gpool.tile([128, GJ * K], DT_F32, name="st")
                nc.vector.tensor_tensor(
                    out=st.rearrange("p (j k) -> p j k", k=K),
                    in0=ex3,
                    in1=sminv[:, :, None].broadcast_to([128, GJ, K]),
                    op=ALU.mult,
                )
                msk = gpool.tile([128, GJ * K], DT_F32, name="msk")
                nc.vector.tensor_tensor(out=msk, in0=st, in1=thr_s, op=ALU.is_gt)
                nc.sync.dma_start(out=mo.ap(), in_=msk)
                gg = gpool.tile([128, GJ * K], DT_F32, name="gg")
                nc.vector.tensor_mul(out=gg, in0=st, in1=msk)
                gs = gpool.tile([128, GJ], DT_F32, name="gs")
                nc.vector.tensor_reduce(
                    out=gs,
                    in_=gg.rearrange("p (j k) -> p j k", k=K),
                    axis=AX.X,
                    op=ALU.add,
                )
                gsel = gpool.tile([128, GJ * K], DT_F32, name="gsel")
                nc.vector.tensor_mul(out=gsel, in0=gg, in1=sel_s)
                wnum = gpool.tile([128, GJ], DT_F32, name="wnum")
                nc.vector.tensor_reduce(
                    out=wnum,
                    in_=gsel.rearrange("p (j k) -> p j k", k=K),
                    axis=AX.X,
                    op=ALU.add,
                )
                dn = gpool.tile([128, GJ], DT_F32, name="dn")
                nc.vector.tensor_scalar_add(out=dn, in0=gs, scalar1=1e-6)
                dninv = gpool.tile([128, GJ], DT_F32, name="dninv")
                nc.vector.reciprocal(out=dninv, in_=dn)
                wsel = gpool.tile([128, GJ], DT_F32, name="wsel")
                nc.vector.tensor_mul(out=wsel, in0=wnum, in1=dninv)
                _ffn(wsel)

            def _ffn(wsel):
                # ---- FFN: per 512-token tile, mm1+gelu then mm2+scale ----
                xtb_v = xtb.ap().rearrange("(c p) t -> p c t", p=128)
                w1_v = w1.ap().rearrange("(c p) f -> p c f", p=128)
                for ti in range(NT):
                    xb = xbpool.tile([128, DC, TOK], DT_BF16, tag="xb", name="xb")
                    nc.sync.dma_start(out=xb, in_=xtb_v[:, :, ds(ti * TOK, TOK)])
                    hts = []
                    for f in range(FC):
                        w1t = w1pool.tile(
                            [128, DC, 128], DT_BF16, tag="w1t", name="w1t"
                        )
                        nc.sync.dma_start(out=w1t, in_=w1_v[:, :, ts(f, 128)])
                        ph = pspool.tile([128, TOK], DT_F32, tag="ph", name="ph")
                        for c in range(DC):
                            nc.tensor.matmul(
                                ph,
                                lhsT=w1t[:, c, :],
                                rhs=xb[:, c, :],
                                start=(c == 0),
                                stop=(c == DC - 1),
                            )
                        ht = hpool.tile(
                            [128, TOK], DT_BF16, tag=f"ht{f}", name=f"ht{f}"
                        )
                        nc.scalar.activation(
                            out=ht,
                            in_=ph,
                            func=AF.Gelu,
                            bias=b1s[:, f : f + 1],
                            scale=1.0,
                        )
                        hts.append(ht)
                    for tj in range(TOK // 128):
                        j = ti * (TOK // 128) + tj
                        po0 = popool.tile([128, 512], DT_F32, tag="po0", name="po0")
                        po1 = popool.tile([128, 512], DT_F32, tag="po1", name="po1")
                        for f in range(FC):
                            lh = hts[f][:, ts(tj, 128)]
                            nc.tensor.matmul(
                                po0,
                                lhsT=lh,
                                rhs=w2s[f][:, 0:512],
                                start=(f == 0),
                                stop=(f == FC - 1),
                            )
                            nc.tensor.matmul(
                                po1,
                                lhsT=lh,
                                rhs=w2s[f][:, 512:1024],
                                start=(f == 0),
                                stop=(f == FC - 1),
                            )
                        yt0 = ypool.tile([128, 512], DT_F32, tag="yt0", name="yt0")
                        nc.vector.tensor_scalar_mul(
                            out=yt0, in0=po0, scalar1=wsel[:, j : j + 1]
                        )
                        nc.sync.dma_start(
                            out=y.ap()[ds(j * 128, 128), 0:512], in_=yt0
                        )
                        yt1 = ypool.tile([128, 512], DT_F32, tag="yt1", name="yt1")
                        nc.vector.tensor_scalar_mul(
                            out=yt1, in0=po1, scalar1=wsel[:, j : j + 1]
                        )
                        nc.sync.dma_start(
                            out=y.ap()[ds(j * 128, 128), 512:1024], in_=yt1
                        )

            if repeat == 1:
                body()
            else:
                with tc.For_i(0, repeat, 1):
                    body()

    nc.compile()
    return nc


_NC_CACHE = None


def _get_nc():
    global _NC_CACHE
    if _NC_CACHE is None:
        _NC_CACHE = _build(repeat=int(os.environ.get("KERNEL_REPEAT", "1")))
    return _NC_CACHE


def _prep_in_maps(x, prev_logits, Wt, Wgt, We_logits, W1, b1, W2, b2):
    # ---- host-side shard prep ----
    x2 = x.reshape(BT, D)
    xtf = np.ascontiguousarray(x2.T)
    xtb = np.ascontiguousarray(xtf.astype(BF16))
    pt = np.ascontiguousarray(prev_logits.reshape(BT, K).T)
    th = (F32(LAMBDA) / (F32(1.0) + np.exp(-We_logits, dtype=F32))).astype(F32)
    thr_tiled = np.ascontiguousarray(np.tile(th, (128, GJ)))

    shared = {
        "xtf": xtf,
        "xtb": xtb,
        "pt": pt,
        "wt": Wt,
        "wgt": Wgt,
        "thr": thr_tiled,
    }
    in_maps = []
    for k in range(NCORES):
        e = np.zeros(K, F32)
        e[k] = 1.0
        in_maps.append(
            dict(
                shared,
                sel=np.ascontiguousarray(np.tile(e, (128, GJ))),
                w1=np.ascontiguousarray(W1[k].astype(BF16)),
                b1t=np.ascontiguousarray(b1[k].reshape(FC, 128).T),
                w2=np.ascontiguousarray(W2[k].astype(BF16)),
            )
        )
    return in_maps


def _postprocess(results, b2):
    # ---- host-side unshard ----
    y = results[0]["y"].astype(F32)
    for k in range(1, NCORES):
        y = y + results[k]["y"]

    def untile(a):  # [128, GJ*K] -> [BT, K]
        return a.reshape(128, GJ, K).transpose(1, 0, 2).reshape(BT, K)

    logits = untile(results[0]["lo"])
    mask = untile(results[0]["mo"])

    # b2 enters linearly through the gating weights; add it on the host.
    e = np.exp(logits - logits.max(-1, keepdims=True), dtype=F32)
    st = e / e.sum(-1, keepdims=True)
    g = st * mask
    w = g / (g.sum(-1, keepdims=True) + F32(1e-6))
    y = y + w @ b2

    sel_counts = mask.sum(axis=0, dtype=F32)
    avg_density = F32(sel_counts.sum() / F32(BT))

    return (
        y.reshape(B, T, D),
        logits.reshape(B, T, K),
        sel_counts,
        np.asarray(avg_density, F32),
    )


def _enable_jax_cache():
    try:
        import jax

        jax.config.update("jax_compilation_cache_dir", "/tmp/jax_kernel_cache")
        jax.config.update("jax_persistent_cache_min_compile_time_secs", 1.0)
    except Exception:
        pass


def kernel(x, prev_logits, Wt, Wgt, We_logits, W1, b1, W2, b2):
    global LAST_RESULT
    _enable_jax_cache()
    x = np.asarray(x, F32)
    prev_logits = np.asarray(prev_logits, F32)
    Wt = np.asarray(Wt, F32)
    Wgt = np.asarray(Wgt, F32)
    We_logits = np.asarray(We_logits, F32)
    W1 = np.asarray(W1, F32)
    b1 = np.asarray(b1, F32)
    W2 = np.asarray(W2, F32)
    b2 = np.asarray(b2, F32)

    in_maps = _prep_in_maps(x, prev_logits, Wt, Wgt, We_logits, W1, b1, W2, b2)
    nc = _get_nc()
    res = run_bass_kernel_spmd(nc, in_maps, core_ids=list(range(NCORES)))
    LAST_RESULT = res
    return _postprocess(res.results, b2)
